# revision 1
# baseline (speedup 1.0000x reference)
"""GroupRouter MoE routing kernel for 8 Trainium2 NeuronCores.

Problem: B=262144 tokens, D=512 features, G=4 groups x GS=4 experts, top-2.
  group_logits = x @ group_w.T + group_b            [B, 4]
  top_group    = argmax(group_logits)               [B]
  in_logits    = x @ in_w[top_group].T + in_b[..]   [B, 4]
  probs        = softmax(in_logits)                 [B, 4]
  (weights, in_idx) = top2(probs); expert = experts_table[top_group, in_idx]

Strategy: data-parallel over 8 cores (32768 tokens each). Per core, one fused
GEMM x @ Wc.T with Wc = [group_w; in_w.reshape(16,512)] -> 20 logits/token.
x tiles are PE-transposed on chip (matmul contracts the partition dim, so x
must present d on partitions), the 20-row result is transposed back so tokens
sit on partitions, and the argmax/softmax/top-2 runs as segmented vector ops
over all 32768 tokens of the core at once.  token = p*256 + s (p: partition,
s: slot) so all DMAs are 2KB-contiguous per partition.
"""

import numpy as np

import concourse.bacc as bacc
import concourse.tile as tile
import concourse.mybir as mybir
from concourse.bass_utils import run_bass_kernel_spmd

B, D, G, GS = 262144, 512, 4, 4
NO = G + G * GS            # 20 logit rows (4 group + 16 in-group)
NCORES = 8
BC = B // NCORES           # 32768 tokens per core
P = 128                    # SBUF partitions
S = BC // P                # 256 slots per partition; token = p*S + s
NBLK = S // 4              # 64 DMA blocks of 4 slots (1MB each)
NHALF = S // 2             # 128 compute half-blocks of 2 slots (256 tokens)
CH = D // P                # 4 contraction chunks of 128

F32 = mybir.dt.float32
I32 = mybir.dt.int32
AX = mybir.AxisListType
OP = mybir.AluOpType

_cached_nc = None


def _copy(use_vector, nc, out, in_):
    if use_vector:
        nc.vector.tensor_copy(out, in_)
    else:
        nc.scalar.copy(out, in_)


def _build():
    nc = bacc.Bacc("TRN2", target_bir_lowering=False, num_devices=NCORES)
    x = nc.dram_tensor("x", [BC, D], F32, kind="ExternalInput")
    wt = nc.dram_tensor("wt", [D, NO], F32, kind="ExternalInput")
    bias = nc.dram_tensor("bias", [NO], F32, kind="ExternalInput")
    idx_o = nc.dram_tensor("idx_o", [BC, 2], I32, kind="ExternalOutput")
    w_o = nc.dram_tensor("w_o", [BC, 2], F32, kind="ExternalOutput")

    x_r = x.ap().rearrange("(p s) d -> p s d", p=P)          # [128, 256, 512]
    idx_r = idx_o.ap().rearrange("(p s) k -> p s k", p=P)    # [128, 256, 2]
    w_r = w_o.ap().rearrange("(p s) k -> p s k", p=P)

    with tile.TileContext(nc) as tc:
        with (
            tc.tile_pool(name="singles", bufs=1) as singles,
            tc.tile_pool(name="xs_pool", bufs=3) as xs_pool,
            tc.tile_pool(name="xt_pool", bufs=3) as xt_pool,
            tc.tile_pool(name="lg_pool", bufs=3) as lg_pool,
            tc.tile_pool(name="big", bufs=1) as big,
            tc.tile_pool(name="post", bufs=1) as post,
            tc.tile_pool(name="pxt_pool", bufs=2, space="PSUM") as pxt_pool,
            tc.tile_pool(name="plg_pool", bufs=2, space="PSUM") as plg_pool,
            tc.tile_pool(name="pt2_pool", bufs=2, space="PSUM") as pt2_pool,
        ):
            # ---- constants ----
            ident = singles.tile([P, P], F32)
            nc.vector.memset(ident, 1.0)
            nc.gpsimd.affine_select(
                ident, ident, pattern=[[-1, P]], base=0, channel_multiplier=1,
                compare_op=OP.is_equal, fill=0.0)
            wt_sb = singles.tile([P, CH, NO], F32)
            nc.sync.dma_start(out=wt_sb, in_=wt.ap().rearrange("(c p) j -> p c j", p=P))
            bias_sb = singles.tile([1, NO], F32)
            nc.sync.dma_start(out=bias_sb, in_=bias.ap().unsqueeze(0))
            ones = singles.tile([1, 2 * P], F32)
            nc.vector.memset(ones, 1.0)
            kconst = singles.tile([P, GS], F32)
            k4const = singles.tile([P, G], F32)
            for j in range(GS):
                nc.vector.memset(kconst[:, j:j + 1], float(j))
                nc.vector.memset(k4const[:, j:j + 1], float(j * GS))

            # per-token logits, token-major: [p, half, i, j]
            L = big.tile([P, NHALF, 2, NO], F32)

            # ---- main loop: 64 DMA blocks x 2 half-blocks ----
            for blk in range(NBLK):
                xs = xs_pool.tile([P, 4, D], F32)
                nc.sync.dma_start(out=xs, in_=x_r[:, 4 * blk:4 * blk + 4, :])
                for hh in range(2):
                    h = 2 * blk + hh
                    # transpose 2 slots x 4 chunks -> psum [128, 1024]
                    pxt = pxt_pool.tile([P, 2 * D], F32)
                    for i in range(2):
                        for c in range(CH):
                            off = (c // 2) * 512 + (c % 2) * 256 + i * P
                            nc.tensor.transpose(
                                pxt[:, off:off + P],
                                xs[:, 2 * hh + i, c * P:(c + 1) * P], ident)
                    xt = xt_pool.tile([P, CH, 2 * P], F32)
                    for k in range(2):
                        _copy((h + k) % 2 == 0, nc,
                              xt[:, 2 * k:2 * k + 2, :].rearrange("p c n -> p (c n)"),
                              pxt[:, 512 * k:512 * (k + 1)])
                    # fused GEMM: 4 chunk matmuls + bias row, accumulate [20, 256]
                    plg = plg_pool.tile([P, 2 * P], F32)
                    for c in range(CH):
                        nc.tensor.matmul(plg[0:NO, :], wt_sb[:, c, :], xt[:, c, :],
                                         start=(c == 0), stop=False)
                    nc.tensor.matmul(plg[0:NO, :], bias_sb, ones,
                                     start=False, stop=True)
                    lgs = lg_pool.tile([NO, 2 * P], F32)
                    _copy(h % 2 == 0, nc, lgs, plg[0:NO, :])
                    # transpose result back: tokens on partitions
                    pt2 = pt2_pool.tile([P, 2, NO], F32)
                    for i in range(2):
                        nc.tensor.transpose(pt2[:, i, :], lgs[:, i * P:(i + 1) * P],
                                            ident[0:NO, 0:NO])
                    _copy(h % 2 == 1, nc, L[:, h, :, :], pt2)

            # ---- postprocess: all 32768 tokens at once ----
            LL = L[:, :, :, :].rearrange("p h i j -> p (h i) j")   # [128, 256, 20]
            Gv = LL[:, :, 0:G]
            INv = LL[:, :, G:NO].rearrange("p s (g k) -> p s g k", g=G)

            def bcast(t):  # [128, 256] -> [128, 256, 4] (stride-0 inner)
                return t[:, :].unsqueeze(2).broadcast_to([P, S, 4])

            gmax = post.tile([P, S], F32)
            nc.vector.tensor_reduce(gmax, Gv, axis=AX.X, op=OP.max)
            eqg = post.tile([P, S, G], F32)
            nc.vector.tensor_tensor(eqg, Gv, bcast(gmax), op=OP.is_equal)
            # select chosen group's 4 in-logits: sum_g eq[g] * in[g, k]
            tmp = post.tile([P, S, GS, G], F32)      # storage [s, k, g]
            nc.vector.tensor_tensor(
                tmp.rearrange("p s k g -> p s g k"),
                eqg.unsqueeze(3).broadcast_to([P, S, G, GS]), INv, op=OP.mult)
            sel = post.tile([P, S, GS], F32)
            nc.vector.tensor_reduce(sel, tmp, axis=AX.X, op=OP.add)
            # softmax over the 4 selected logits
            e = post.tile([P, S, GS], F32)
            nc.scalar.activation(e, sel, func=mybir.ActivationFunctionType.Exp)
            ssum = post.tile([P, S], F32)
            nc.vector.tensor_reduce(ssum, e, axis=AX.X, op=OP.add)
            rcp = post.tile([P, S], F32)
            nc.vector.reciprocal(rcp, ssum)
            pr = post.tile([P, S, GS], F32)
            nc.vector.tensor_tensor(pr, e, bcast(rcp), op=OP.mult)
            # top-2 values + in-group indices
            wout = post.tile([P, S, 2], F32)
            p1 = wout[:, :, 0]
            nc.vector.tensor_reduce(p1, pr, axis=AX.X, op=OP.max)
            eq1 = post.tile([P, S, GS], F32)
            nc.vector.tensor_tensor(eq1, pr, bcast(p1), op=OP.is_equal)
            tk = post.tile([P, S, GS], F32)
            kb = kconst.unsqueeze(1).broadcast_to([P, S, GS])
            nc.vector.tensor_tensor(tk, eq1, kb, op=OP.mult)
            i1 = post.tile([P, S], F32)
            nc.vector.tensor_reduce(i1, tk, axis=AX.X, op=OP.add)
            pm = post.tile([P, S, GS], F32)
            nc.vector.scalar_tensor_tensor(pm, eq1, -1e30, pr,
                                           op0=OP.mult, op1=OP.add)
            p2 = wout[:, :, 1]
            nc.vector.tensor_reduce(p2, pm, axis=AX.X, op=OP.max)
            eq2 = post.tile([P, S, GS], F32)
            nc.vector.tensor_tensor(eq2, pm, bcast(p2), op=OP.is_equal)
            tk2 = post.tile([P, S, GS], F32)
            nc.vector.tensor_tensor(tk2, eq2, kb, op=OP.mult)
            i2 = post.tile([P, S], F32)
            nc.vector.tensor_reduce(i2, tk2, axis=AX.X, op=OP.add)
            # group base index (4*g) from the group-argmax mask
            tg = post.tile([P, S, G], F32)
            nc.vector.tensor_tensor(tg, eqg,
                                    k4const.unsqueeze(1).broadcast_to([P, S, G]),
                                    op=OP.mult)
            g4 = post.tile([P, S], F32)
            nc.vector.tensor_reduce(g4, tg, axis=AX.X, op=OP.add)
            iout = post.tile([P, S, 2], I32)
            nc.vector.tensor_tensor(iout[:, :, 0], g4, i1, op=OP.add)
            nc.vector.tensor_tensor(iout[:, :, 1], g4, i2, op=OP.add)
            nc.sync.dma_start(out=idx_r, in_=iout)
            nc.sync.dma_start(out=w_r, in_=wout)
    nc.finalize()
    return nc


def _get_nc():
    global _cached_nc
    if _cached_nc is None:
        _cached_nc = _build()
    return _cached_nc


def kernel(routing_features, group_w, group_b, in_w, in_b, experts_table,
           trace=False):
    x = np.ascontiguousarray(np.asarray(routing_features, np.float32))
    gw = np.asarray(group_w, np.float32)
    gb = np.asarray(group_b, np.float32)
    iw = np.asarray(in_w, np.float32).reshape(G * GS, D)
    ib = np.asarray(in_b, np.float32).reshape(G * GS)
    table = np.asarray(experts_table, np.int32).reshape(-1)

    wt = np.ascontiguousarray(np.concatenate([gw, iw], 0).T)   # [512, 20]
    bias = np.concatenate([gb, ib], 0)                          # [20]

    shards = x.reshape(NCORES, BC, D)
    in_maps = [{"x": shards[c], "wt": wt, "bias": bias} for c in range(NCORES)]
    try:
        res = run_bass_kernel_spmd(_get_nc(), in_maps,
                                   core_ids=list(range(NCORES)), trace=trace)
    except (ImportError, ModuleNotFoundError):
        # NTFF profiling hook unavailable in this environment; run untraced.
        res = run_bass_kernel_spmd(_get_nc(), in_maps,
                                   core_ids=list(range(NCORES)), trace=False)
    idx = np.concatenate([res.results[c]["idx_o"] for c in range(NCORES)], 0)
    w = np.concatenate([res.results[c]["w_o"] for c in range(NCORES)], 0)
    expert_indices = table[idx]                                 # int32 [B, 2]
    if trace:
        kernel.last_exec_time_ns = res.exec_time_ns
        if kernel.last_exec_time_ns is None:
            # No hardware NTFF in this environment: fall back to the
            # CoreSim cost-model timeline (single core, SPMD-identical).
            try:
                from concourse.timeline_sim import TimelineSim
                kernel.last_exec_time_ns = int(TimelineSim(_get_nc()).simulate())
                kernel.time_source = "cost-model timeline sim"
            except Exception:
                pass
        else:
            kernel.time_source = "ntff"
    return expert_indices, w

